# revision 1
# baseline (speedup 1.0000x reference)
"""DeepSeekV2 MoE layer on 8 NeuronCores (Trainium2, raw Bass).

Strategy (expert parallelism, full-I/O contract):
  - Host: router (gate matmul + softmax + top-6, renormalized) in float64,
    dispatch tokens per expert (the "all-to-all" is a host-side gather under
    this full-input/full-output contract), pad each expert to capacity C.
  - Device (8 cores, SPMD, 2 experts/core): per expert
        h = silu(x @ w1g.T) * (x @ w1u.T);  y = h @ w2.T
    as f32r (full-rate fp32) matmuls on the PE, silu on ACT, combine-mul and
    PSUM eviction on DVE. Raw Bass with explicit semaphores (standalone
    wait_ge instructions — this toolchain only allows 1 embedded wait per
    fp32-matmul/DMA instruction, so Tile-generated sync cannot compile).
  - Host: weighted scatter-add of expert outputs (topk weights).

DMA sync: one semaphore per double-buffered SBUF slot, so each sem has at
most one DMA in flight (cumulative 16*n thresholds are unsafe when several
DMAs share a sem — the 16 per-engine increments interleave).

Device layouts (host pre-packs so every DMA is a clean 3D access pattern):
  xT   [EPC, D, C]        tokens gathered per expert, transposed (d-major)
  w13s [EPC, NI, D, 2P]   per i-tile: [d, gate(P) | up(P)]  (transposed)
  w2s  [EPC, ND, I, P]    per d-tile: [i, d(P)]             (transposed)
  y    [EPC, D, C]        expert outputs (d-major)
"""

import math
import os
import numpy as np
from contextlib import ExitStack

P = 128
N_CORES = 8

# test/bench hook: set by kernel() after each run (ns, or None if untraced)
LAST_EXEC_NS = None


# --------------------------------------------------------------------------
# Device program
# --------------------------------------------------------------------------

def emit_moe(nc, xT, w13s, w2s, y, EPC, D, I, C, nblk, repeat=1):
    """Emit the per-core MoE program (raw Bass, explicit sems).

    repeat>1 re-runs the whole expert sequence (benchmarking only — later
    passes overwrite y with identical values).
    """
    import concourse.mybir as mybir

    f32 = mybir.dt.float32
    KD = D // P          # k-tiles over hidden dim (gate/up contraction)
    NI = I // P          # i-tiles over intermediate dim
    ND = D // P          # d-tiles of the output
    NB = C // nblk       # token-block (matmul free dim)
    assert C % nblk == 0 and D % P == 0 and I % P == 0
    seq = [e for _ in range(repeat) for e in range(EPC)]
    NSEQ = len(seq)

    # ---- semaphore tick pre-computation -----------------------------------
    # s_pe: +1 after each matmul group (pg / pu / py)
    T_pg, T_pu, T_py = {}, {}, {}
    t = 0
    for e in range(NSEQ):
        for i in range(NI):
            for cb in range(nblk):
                t += 1; T_pg[e, i, cb] = t
                t += 1; T_pu[e, i, cb] = t
        for d in range(ND):
            for cb in range(nblk):
                t += 1; T_py[e, d, cb] = t

    # s_act: +1 after each silu
    T_ss = {}
    t = 0
    for e in range(NSEQ):
        for i in range(NI):
            for cb in range(nblk):
                t += 1; T_ss[e, i, cb] = t

    # s_dve: +2 per phase-1 group (both h-muls inc; the DVE pipeline does not
    # order back-to-back dependent ops, so the second mul waits on the first),
    # +1 per y-copy
    T_h, T_yc = {}, {}
    t = 0
    for e in range(NSEQ):
        for i in range(NI):
            for cb in range(nblk):
                t += 2; T_h[e, i, cb] = t
        for d in range(ND):
            for cb in range(nblk):
                t += 1; T_yc[e, d, cb] = t

    W13S = min(4, NI)     # w13 ring depth (prefetch across the phase boundary)
    W2S = min(3, ND)      # w2 ring depth
    W2PRE = min(3, ND)    # w2 tiles prefetched into the phase-1 window
    KH = max(1, KD // 2)  # x A-half k-tiles (double-buffered, early prefetch)

    # per-slot DMA load indices (nth load of that slot, 1-based)
    w13_load, w2_load, dy_idx = {}, {}, {}
    cnt13 = [0] * W13S
    cnt2 = [0] * W2S
    cnty = [0, 0]
    for e in range(NSEQ):
        for i in range(NI):
            s = i % W13S
            cnt13[s] += 1
            w13_load[e, i] = cnt13[s]
        for d in range(ND):
            s = d % W2S
            cnt2[s] += 1
            w2_load[e, d] = cnt2[s]
        for d in range(ND):
            for cb in range(nblk):
                s = (d * nblk + cb) % 2
                cnty[s] += 1
                dy_idx[e, d, cb] = cnty[s]

    # SP issue order. Phase 2 is the DMA-tight window: x's A-half (k<KH) is
    # double-buffered and prefetches during the PREVIOUS phase 1 (DMA-light);
    # the B-half + first w13 tiles interleave with the w2 stream in phase 2;
    # w2[e][0..W2PRE) prefetch into late phase 1.
    dma_seq = []
    for e in range(NSEQ):
        if e == 0:
            dma_seq += [("xa", 0, 0), ("w13", 0, 0)]
            dma_seq += [("xa", 0, cb) for cb in range(1, nblk)]
            dma_seq += [("xb", 0, cb) for cb in range(nblk)]
            dma_seq += [("w13", 0, i) for i in range(1, min(2, NI))]
        else:
            pre = [("xb", e, cb) for cb in range(nblk)]
            pre += [("w13", e, i) for i in range(min(2, NI))]
            tail_w2 = [("w2", e - 1, d) for d in range(W2PRE, ND)]
            # 2 w2 tiles per prefetch item: w2 consumption (3.7us/tile) must
            # not fall behind the larger x/w13 prefetch transfers
            merged = []
            rnd = 0
            while pre or tail_w2:
                if tail_w2:
                    merged.append(tail_w2.pop(0))
                if rnd % 2 == 1 and pre:
                    merged.append(pre.pop(0))
                if not tail_w2 and pre:
                    merged.append(pre.pop(0))
                rnd += 1
            dma_seq += merged
        dma_seq += [("w13", e, i) for i in range(min(2, NI), NI)]
        if e + 1 < NSEQ:
            dma_seq += [("xa", e + 1, cb) for cb in range(nblk)]
        dma_seq += [("w2", e, d) for d in range(W2PRE)]
    dma_seq += [("w2", NSEQ - 1, d) for d in range(W2PRE, ND)]

    with ExitStack() as ctx:
        f32r = mybir.dt.float32r
        xbufA = [
            ctx.enter_context(nc.sbuf_tensor(f"xbufA{s}", [P, KH, C], f32r))
            for s in range(2)
        ]
        xbufB = ctx.enter_context(
            nc.sbuf_tensor("xbufB", [P, KD - KH, C], f32r)
        )
        hbuf = ctx.enter_context(nc.sbuf_tensor("hbuf", [P, NI, C], f32r))
        w13b = [
            ctx.enter_context(nc.sbuf_tensor(f"w13b{s}", [P, KD, 2 * P], f32r))
            for s in range(W13S)
        ]
        w2b = [
            ctx.enter_context(nc.sbuf_tensor(f"w2b{s}", [P, NI, P], f32r))
            for s in range(W2S)
        ]
        ssb = [
            ctx.enter_context(nc.sbuf_tensor(f"ssb{s}", [P, NB], f32))
            for s in range(2)
        ]
        ysb = [
            ctx.enter_context(nc.sbuf_tensor(f"ysb{s}", [P, NB], f32))
            for s in range(2)
        ]
        pg = [
            ctx.enter_context(nc.psum_tensor(f"pg{s}", [P, NB], f32))
            for s in range(2)
        ]
        pu = [
            ctx.enter_context(nc.psum_tensor(f"pu{s}", [P, NB], f32))
            for s in range(2)
        ]
        py = [
            ctx.enter_context(nc.psum_tensor(f"py{s}", [P, NB], f32))
            for s in range(2)
        ]
        s_pe = ctx.enter_context(nc.semaphore("s_pe"))
        s_act = ctx.enter_context(nc.semaphore("s_act"))
        s_dve = ctx.enter_context(nc.semaphore("s_dve"))
        d_xa = [
            ctx.enter_context(nc.semaphore(f"d_xa{s}"))
            for s in range(2 * nblk)
        ]
        d_xb = [
            ctx.enter_context(nc.semaphore(f"d_xb{cb}")) for cb in range(nblk)
        ]
        d_w13 = [
            ctx.enter_context(nc.semaphore(f"d_w13{s}")) for s in range(W13S)
        ]
        d_w2 = [
            ctx.enter_context(nc.semaphore(f"d_w2{s}")) for s in range(W2S)
        ]
        d_y = [ctx.enter_context(nc.semaphore(f"d_y{s}")) for s in range(2)]

        block = ctx.enter_context(nc.Block(no_gpsimd_drain=True))

        # ---------------- SP: all input DMAs (FIFO issue ring) -------------
        @block.sync
        def _(sync):
            w13_last = [0] * W13S  # s_pe tick of previous reader of the slot
            w2_last = [0] * W2S
            for key in dma_seq:
                kind, e, j = key
                ex = seq[e]
                if kind == "xa":
                    if e >= 2 and j == 0:
                        sync.wait_ge(s_pe, T_pu[e - 2, NI - 1, nblk - 1])
                    sync.dma_start(
                        xbufA[e % 2][:, :, j * NB : (j + 1) * NB],
                        xT[ex, : KH * P, j * NB : (j + 1) * NB].rearrange(
                            "(ko p) c -> p ko c", p=P
                        ),
                    ).then_inc(d_xa[(e % 2) * nblk + j], 16)
                elif kind == "xb":
                    if e >= 1 and j == 0:
                        sync.wait_ge(s_pe, T_pu[e - 1, NI - 1, nblk - 1])
                    sync.dma_start(
                        xbufB[:, :, j * NB : (j + 1) * NB],
                        xT[ex, KH * P :, j * NB : (j + 1) * NB].rearrange(
                            "(ko p) c -> p ko c", p=P
                        ),
                    ).then_inc(d_xb[j], 16)
                elif kind == "w13":
                    s = j % W13S
                    if w13_last[s]:
                        sync.wait_ge(s_pe, w13_last[s])
                    w13_last[s] = T_pu[e, j, nblk - 1]
                    sync.dma_start(
                        w13b[s][:],
                        w13s[ex, j].rearrange("(ko p) g -> p ko g", p=P),
                    ).then_inc(d_w13[s], 16)
                else:
                    s = j % W2S
                    if w2_last[s]:
                        sync.wait_ge(s_pe, w2_last[s])
                    w2_last[s] = T_py[e, j, nblk - 1]
                    sync.dma_start(
                        w2b[s][:],
                        w2s[ex, j].rearrange("(ko p) g -> p ko g", p=P),
                    ).then_inc(d_w2[s], 16)

        # ---------------- PE: all matmuls ----------------------------------
        @block.tensor
        def _(tensor):
            pg_last = [0, 0]  # s_dve tick of previous reader of pg slot
            pu_last = [0, 0]  # s_dve tick of previous reader of pu slot
            py_last = [0, 0]  # s_dve tick of previous reader of py slot
            for e in range(NSEQ):
                for i in range(NI):
                    for cb in range(nblk):
                        gs = (i * nblk + cb) % 2
                        cs = slice(cb * NB, (cb + 1) * NB)
                        if cb == 0:
                            tensor.wait_ge(
                                d_w13[i % W13S], 16 * w13_load[e, i]
                            )
                        if i == 0:
                            tensor.wait_ge(
                                d_xa[(e % 2) * nblk + cb], 16 * (e // 2 + 1)
                            )
                        if pg_last[gs]:
                            tensor.wait_ge(s_dve, pg_last[gs])
                        pg_last[gs] = T_h[e, i, cb]
                        for k in range(KD):
                            if i == 0 and k == KH:
                                tensor.wait_ge(d_xb[cb], 16 * (e + 1))
                            mm = tensor.matmul(
                                pg[gs][:, :],
                                w13b[i % W13S][:, k, 0:P],
                                xbufA[e % 2][:, k, cs]
                                if k < KH
                                else xbufB[:, k - KH, cs],
                                start=(k == 0),
                                stop=(k == KD - 1),
                            )
                        mm.then_inc(s_pe)
                        if pu_last[gs]:
                            tensor.wait_ge(s_dve, pu_last[gs])
                        pu_last[gs] = T_h[e, i, cb]
                        for k in range(KD):
                            mm = tensor.matmul(
                                pu[gs][:, :],
                                w13b[i % W13S][:, k, P : 2 * P],
                                xbufA[e % 2][:, k, cs]
                                if k < KH
                                else xbufB[:, k - KH, cs],
                                start=(k == 0),
                                stop=(k == KD - 1),
                            )
                        mm.then_inc(s_pe)
                for d in range(ND):
                    for cb in range(nblk):
                        ys = (d * nblk + cb) % 2
                        cs = slice(cb * NB, (cb + 1) * NB)
                        if cb == 0:
                            tensor.wait_ge(d_w2[d % W2S], 16 * w2_load[e, d])
                        if py_last[ys]:
                            tensor.wait_ge(s_dve, py_last[ys])
                        py_last[ys] = T_yc[e, d, cb]
                        for k in range(NI):
                            if d == 0:
                                # start the down-proj as h tiles land
                                tensor.wait_ge(s_dve, T_h[e, k, cb])
                            mm = tensor.matmul(
                                py[ys][:, :],
                                w2b[d % W2S][:, k, :],
                                hbuf[:, k, cs],
                                start=(k == 0),
                                stop=(k == NI - 1),
                            )
                        mm.then_inc(s_pe)

        # ---------------- ACT: silu + output DMAs --------------------------
        @block.scalar
        def _(scalar):
            ss_last = [0, 0]  # s_dve tick of previous reader of ss slot
            for e in range(NSEQ):
                ex = seq[e]
                for i in range(NI):
                    for cb in range(nblk):
                        gs = (i * nblk + cb) % 2
                        scalar.wait_ge(s_pe, T_pg[e, i, cb])
                        if ss_last[gs]:
                            scalar.wait_ge(s_dve, ss_last[gs])
                        ss_last[gs] = T_h[e, i, cb]
                        scalar.activation(
                            ssb[gs][:, :],
                            pg[gs][:, :],
                            mybir.ActivationFunctionType.Sigmoid,
                        ).then_inc(s_act)
                for d in range(ND):
                    for cb in range(nblk):
                        ys = (d * nblk + cb) % 2
                        scalar.wait_ge(s_dve, T_yc[e, d, cb])
                        scalar.dma_start(
                            y[ex, d * P : (d + 1) * P, cb * NB : (cb + 1) * NB],
                            ysb[ys][:, :],
                        ).then_inc(d_y[ys], 16)

        # ---------------- DVE: h-mul + psum eviction -----------------------
        @block.vector
        def _(vector):
            ysb_last = [0, 0]  # d_y load index of previous DMA reading slot
            for e in range(NSEQ):
                for i in range(NI):
                    for cb in range(nblk):
                        gs = (i * nblk + cb) % 2
                        cs = slice(cb * NB, (cb + 1) * NB)
                        vector.wait_ge(s_act, T_ss[e, i, cb])
                        vector.wait_ge(s_pe, T_pu[e, i, cb])
                        # h = (sigmoid(g) * u) * g  — one PSUM operand per op
                        vector.tensor_mul(
                            ssb[gs][:, :], ssb[gs][:, :], pu[gs][:, :]
                        ).then_inc(s_dve)
                        vector.wait_ge(s_dve, T_h[e, i, cb] - 1)
                        vector.tensor_mul(
                            hbuf[:, i, cs], ssb[gs][:, :], pg[gs][:, :]
                        ).then_inc(s_dve)
                for d in range(ND):
                    for cb in range(nblk):
                        ys = (d * nblk + cb) % 2
                        vector.wait_ge(s_pe, T_py[e, d, cb])
                        if ysb_last[ys]:
                            vector.wait_ge(d_y[ys], 16 * ysb_last[ys])
                        ysb_last[ys] = dy_idx[e, d, cb]
                        vector.tensor_copy(
                            ysb[ys][:, :], py[ys][:, :]
                        ).then_inc(s_dve)

    return nc


def build_moe(EPC, D, I, C, nblk, repeat=1):
    import concourse.bass as bass
    import concourse.mybir as mybir

    f32 = mybir.dt.float32
    f32r = mybir.dt.float32r
    NI = I // P
    ND = D // P

    nc = bass.Bass()
    xT = nc.dram_tensor("xT", [EPC, D, C], f32r, kind="ExternalInput")
    w13s = nc.dram_tensor("w13s", [EPC, NI, D, 2 * P], f32r, kind="ExternalInput")
    w2s = nc.dram_tensor("w2s", [EPC, ND, I, P], f32r, kind="ExternalInput")
    y = nc.dram_tensor("y", [EPC, D, C], f32, kind="ExternalOutput")
    emit_moe(nc, xT, w13s, w2s, y, EPC, D, I, C, nblk, repeat=repeat)
    return nc


# --------------------------------------------------------------------------
# Host side
# --------------------------------------------------------------------------

def _route(x, gate_w, top_k):
    """float64 router: softmax over gate logits, top-k (set), renormalize."""
    logits = x.astype(np.float64) @ gate_w.astype(np.float64).T
    logits -= logits.max(axis=-1, keepdims=True)
    p = np.exp(logits)
    p /= p.sum(axis=-1, keepdims=True)
    ids = np.argpartition(-p, top_k - 1, axis=-1)[:, :top_k]  # [T, K]
    w = np.take_along_axis(p, ids, axis=-1)
    w = w / w.sum(axis=-1, keepdims=True)
    return ids, w


def _pack_weights(w13, w2):
    E, twoI, D = w13.shape
    I = twoI // 2
    NI = I // P
    ND = D // P
    g = w13[:, :I, :].reshape(E, NI, P, D).transpose(0, 1, 3, 2)  # [E,NI,D,P]
    u = w13[:, I:, :].reshape(E, NI, P, D).transpose(0, 1, 3, 2)
    w13s = np.concatenate([g, u], axis=-1)  # [E, NI, D, 2P]
    w2s = w2.reshape(E, ND, P, I).transpose(0, 1, 3, 2)  # [E, ND, I, P]
    return np.ascontiguousarray(w13s), np.ascontiguousarray(w2s)


def prepare(hidden_states, gate_w, w13, w2, top_k):
    """Host routing + dispatch + device-layout packing.

    Returns (nc, in_maps, meta) where meta carries what combine() needs.
    """
    x = np.ascontiguousarray(np.asarray(hidden_states, dtype=np.float32))
    gate_w = np.asarray(gate_w, dtype=np.float32)
    w13 = np.asarray(w13, dtype=np.float32)
    w2 = np.asarray(w2, dtype=np.float32)
    K = int(top_k)

    T, D = x.shape
    E = gate_w.shape[0]
    I = w2.shape[2]
    EPC = E // N_CORES

    topk_ids, topk_w = _route(x, gate_w, K)

    # dispatch: group (token, weight) pairs by expert
    flat_e = topk_ids.ravel()
    flat_t = np.repeat(np.arange(T), K)
    flat_w = topk_w.ravel()
    order = np.argsort(flat_e, kind="stable")
    sorted_t = flat_t[order]
    sorted_w = flat_w[order]
    counts = np.bincount(flat_e, minlength=E)
    offs = np.zeros(E + 1, np.int64)
    np.cumsum(counts, out=offs[1:])

    cmax = max(int(counts.max()), 16)
    nblk = max(1, math.ceil(cmax / 512))
    NB = math.ceil(cmax / nblk / 16) * 16
    C = NB * nblk

    xT_all = np.zeros((E, D, C), np.float32)
    for e in range(E):
        idx = sorted_t[offs[e] : offs[e + 1]]
        if len(idx):
            xT_all[e, :, : len(idx)] = x[idx].T
    w13s, w2s = _pack_weights(w13, w2)

    nc = build_moe(EPC, D, I, C, nblk)
    in_maps = [
        {
            "xT": np.ascontiguousarray(xT_all[m * EPC : (m + 1) * EPC]),
            "w13s": np.ascontiguousarray(w13s[m * EPC : (m + 1) * EPC]),
            "w2s": np.ascontiguousarray(w2s[m * EPC : (m + 1) * EPC]),
        }
        for m in range(N_CORES)
    ]
    meta = dict(
        T=T, D=D, E=E, EPC=EPC, C=C, nblk=nblk,
        sorted_t=sorted_t, sorted_w=sorted_w, offs=offs,
    )
    return nc, in_maps, meta


def combine(results, meta):
    """Weighted scatter-add of per-expert outputs back to [T, D]."""
    T, D, E, EPC = meta["T"], meta["D"], meta["E"], meta["EPC"]
    sorted_t, sorted_w, offs = meta["sorted_t"], meta["sorted_w"], meta["offs"]
    out = np.zeros((T, D), np.float32)
    for e in range(E):
        idx = sorted_t[offs[e] : offs[e + 1]]
        if len(idx) == 0:
            continue
        wgt = sorted_w[offs[e] : offs[e + 1]].astype(np.float32)
        ye = results[e // EPC]["y"][e % EPC]  # [D, C]
        out[idx] += (ye[:, : len(idx)] * wgt[None, :]).T
    return out


def kernel(hidden_states, gate_w, w13, w2, top_k):
    from concourse.bass_utils import run_bass_kernel_spmd

    nc, in_maps, meta = prepare(hidden_states, gate_w, w13, w2, top_k)
    trace = bool(int(os.environ.get("MOE_TRACE", "0")))
    try:
        res = run_bass_kernel_spmd(
            nc, in_maps, core_ids=list(range(N_CORES)), trace=trace
        )
    except Exception:
        # one retry — transient NRT device errors (e.g. a wedged core from a
        # previous aborted run) usually clear on re-execution
        import time as _time

        _time.sleep(5)
        res = run_bass_kernel_spmd(
            nc, in_maps, core_ids=list(range(N_CORES)), trace=trace
        )
    global LAST_EXEC_NS
    LAST_EXEC_NS = res.exec_time_ns
    return combine(res.results, meta)



# revision 14
# speedup vs baseline: 1.0688x; 1.0688x over previous
"""DeepSeekV2 MoE layer on 8 NeuronCores (Trainium2, raw Bass).

Strategy (expert parallelism, full-I/O contract):
  - Host: router (gate matmul + softmax + top-6, renormalized) in float64,
    dispatch tokens per expert (the "all-to-all" is a host-side gather under
    this full-input/full-output contract), pad each expert to its slot-class
    capacity.
  - Experts are paired big/small: the 8 highest-count experts form slot 0
    (capacity C0 = max count), the 8 lowest form slot 1 (capacity C1 = max
    count of that class) — every core runs the same 2-slot program, so the
    smaller slot-1 capacity cuts padded columns vs a single global capacity.
  - Device (8 cores, SPMD, 2 expert slots/core): per expert
        h = silu(x @ w1g.T) * (x @ w1u.T);  y = h @ w2.T
    as bf16 matmuls on the PE (fp32 PSUM accumulate), silu on ACT,
    combine-mul and PSUM eviction on DVE. bf16 halves DMA traffic and
    enables fast weight load; PE column rate is the same as f32r.
    Raw Bass with explicit semaphores (standalone wait_ge instructions —
    this toolchain only allows 1 embedded wait per matmul/DMA instruction,
    so Tile-generated sync cannot compile).
  - Host: weighted scatter-add of expert outputs (topk weights), fp32.

DMA sync: one semaphore per double-buffered SBUF slot, so each sem has at
most one DMA in flight (cumulative 16*n thresholds are unsafe when several
DMAs share a sem — the 16 per-engine increments interleave). The prologue
streams expert 0's first token-block k-granularly on 4 cycling sems (a
distance-4 overtake would need a DMA engine to skip 4 consecutive queue
entries — impossible with per-engine FIFO descriptor processing).

Device layouts (host pre-packs so every DMA is a clean 3D access pattern):
  xT   [EPC, D, C0]       tokens gathered per expert, transposed (d-major)
  w13s [EPC, NI, D, 2P]   per i-tile: [d, gate(P) | up(P)]  (transposed)
  w2s  [EPC, ND, I, P]    per d-tile: [i, d(P)]             (transposed)
  y    [EPC, D, C0]       expert outputs (d-major, fp32)
Slot 1 uses the leading C1 columns of the C0-sized xT/y buffers.
"""

import math
import os
import numpy as np
from contextlib import ExitStack

P = 128
N_CORES = 8

# test/bench hook: set by kernel() after each run (ns, or None if untraced)
LAST_EXEC_NS = None


# --------------------------------------------------------------------------
# Device program
# --------------------------------------------------------------------------

def emit_moe(nc, xT, w13s, w2s, y, EPC, D, I, CS, NBS, probe=None):
    """Emit the per-core MoE program (raw Bass, explicit sems).

    CS[e], NBS[e]: per-slot token capacity and matmul free-dim block size.
    probe(engine, tag): optional instrumentation hook (CoreSim analysis only).
    """
    import concourse.mybir as mybir

    if probe is None:
        probe = lambda eng, tag: None

    f32 = mybir.dt.float32
    bf16 = mybir.dt.bfloat16
    KD = D // P          # k-tiles over hidden dim (gate/up contraction)
    NI = I // P          # i-tiles over intermediate dim
    ND = D // P          # d-tiles of the output
    NBLK = [C // NB for C, NB in zip(CS, NBS)]
    assert all(C % NB == 0 for C, NB in zip(CS, NBS))
    assert D % P == 0 and I % P == 0
    NSEQ = EPC
    NBMAX = max(NBS)
    CMAX = max(CS)
    NBLK_TOT = [0] + list(np.cumsum(NBLK))

    # ---- semaphore tick pre-computation -----------------------------------
    # s_pe: +1 after each matmul group (pg / pu / py)
    T_pg, T_pu, T_py = {}, {}, {}
    t = 0
    for e in range(NSEQ):
        for i in range(NI):
            for cb in range(NBLK[e]):
                t += 1; T_pg[e, i, cb] = t
                t += 1; T_pu[e, i, cb] = t
        for d in range(ND):
            for cb in range(NBLK[e]):
                t += 1; T_py[e, d, cb] = t

    # s_act: +1 after each silu
    T_ss = {}
    t = 0
    for e in range(NSEQ):
        for i in range(NI):
            for cb in range(NBLK[e]):
                t += 1; T_ss[e, i, cb] = t

    # s_dve: +2 per phase-1 group (both h-muls inc; the DVE pipeline does not
    # order back-to-back dependent ops, so the second mul waits on the first),
    # +1 per y-copy
    T_h, T_yc = {}, {}
    t = 0
    for e in range(NSEQ):
        for i in range(NI):
            for cb in range(NBLK[e]):
                t += 2; T_h[e, i, cb] = t
        for d in range(ND):
            for cb in range(NBLK[e]):
                t += 1; T_yc[e, d, cb] = t

    W13S = min(4, NI)     # w13 ring depth (prefetch across the phase boundary)
    W2S = min(4, ND)      # w2 ring depth
    W2PRE = min(3, ND)    # w2 tiles prefetched into the phase-1 window
    KH = max(1, KD // 2)  # x A-half k-tiles (double-buffered, early prefetch)
    NQ = 4                # w13 tile-0 prologue quarters / x prologue sems
    XCH = 2               # k-tiles per x prologue chunk
    NXCH = KD // XCH

    # SP issue order. Phase 2 is the DMA-tight window: x's A-half (k<KH) is
    # double-buffered and prefetches during the PREVIOUS phase 1 (DMA-light);
    # the B-half + first w13 tiles interleave with the w2 stream in phase 2;
    # w2[e][0..W2PRE) prefetch into late phase 1.
    # Expert 0's first token block streams k-granularly on SP while ACT
    # issues w13 tile 0 as gate/up quarters in parallel, so the PE starts
    # within ~3us instead of waiting for whole-tensor transfers.
    dma_seq = []
    for e in range(NSEQ):
        if e == 0:
            dma_seq += [("xk", 0, j) for j in range(NXCH)]
            dma_seq += [("xa", 0, cb) for cb in range(1, NBLK[0])]
            dma_seq += [("xb", 0, cb) for cb in range(1, NBLK[0])]
            dma_seq += [("w13", 0, 1)]
        else:
            pre = [("xb", e, cb) for cb in range(NBLK[e])]
            pre += [("w13", e, i) for i in range(min(2, NI))]
            tail_w2 = [("w2", e - 1, d) for d in range(W2PRE, ND)]
            # 2 w2 tiles per prefetch item: w2 consumption must not fall
            # behind the larger x/w13 prefetch transfers
            merged = []
            rnd = 0
            while pre or tail_w2:
                if tail_w2:
                    merged.append(tail_w2.pop(0))
                if rnd % 2 == 1 and pre:
                    merged.append(pre.pop(0))
                if not tail_w2 and pre:
                    merged.append(pre.pop(0))
                rnd += 1
            dma_seq += merged
        dma_seq += [("w13", e, i) for i in range(min(2, NI), NI)]
        if e + 1 < NSEQ:
            dma_seq += [("xa", e + 1, cb) for cb in range(NBLK[e + 1])]
        dma_seq += [("w2", e, d) for d in range(W2PRE)]
    dma_seq += [("w2", NSEQ - 1, d) for d in range(W2PRE, ND)]

    # per-(kind,e,j) completion sem + threshold, assigned in issue order
    dma_done = {}
    sem_count = {}

    def assign(key, sem_name):
        n = sem_count.get(sem_name, 0) + 1
        sem_count[sem_name] = n
        dma_done[key] = (sem_name, 16 * n)

    # w13 tile 0 quarters issued from the ACT queue (gate then up half)
    for q in range(NQ):
        assign(("w13g", 0, q), f"d_wg{q}")
    for q in range(NQ):
        assign(("w13u", 0, q), f"d_wu{q}")

    for key in dma_seq:
        kind, e, j = key
        if kind == "xk":
            assign(key, f"d_xk{j}")  # one sem per prologue chunk (no reuse)
        elif kind == "xa":
            assign(key, f"d_xa{(e % 2) * max(NBLK) + j}")
        elif kind == "xb":
            assign(key, f"d_xb{j}")
        elif kind == "w13":
            assign(key, f"d_w13{j % W13S}")
        else:
            assign(key, f"d_w2{j % W2S}")

    with ExitStack() as ctx:
        xbufA = [
            ctx.enter_context(nc.sbuf_tensor(f"xbufA{s}", [P, KH, CMAX], bf16))
            for s in range(2)
        ]
        xbufB = ctx.enter_context(
            nc.sbuf_tensor("xbufB", [P, KD - KH, CMAX], bf16)
        )
        hbuf = ctx.enter_context(nc.sbuf_tensor("hbuf", [P, NI, CMAX], bf16))
        w13b = [
            ctx.enter_context(nc.sbuf_tensor(f"w13b{s}", [P, KD, 2 * P], bf16))
            for s in range(W13S)
        ]
        w2b = [
            ctx.enter_context(nc.sbuf_tensor(f"w2b{s}", [P, NI, P], bf16))
            for s in range(W2S)
        ]
        ssb = [
            ctx.enter_context(nc.sbuf_tensor(f"ssb{s}", [P, NBMAX], f32))
            for s in range(2)
        ]
        ysb = [
            ctx.enter_context(nc.sbuf_tensor(f"ysb{s}", [P, NBMAX], f32))
            for s in range(2)
        ]
        pg = [
            ctx.enter_context(nc.psum_tensor(f"pg{s}", [P, NBMAX], f32))
            for s in range(2)
        ]
        pu = [
            ctx.enter_context(nc.psum_tensor(f"pu{s}", [P, NBMAX], f32))
            for s in range(2)
        ]
        py = [
            ctx.enter_context(nc.psum_tensor(f"py{s}", [P, NBMAX], f32))
            for s in range(2)
        ]
        s_pe = ctx.enter_context(nc.semaphore("s_pe"))
        s_act = ctx.enter_context(nc.semaphore("s_act"))
        s_dve = ctx.enter_context(nc.semaphore("s_dve"))
        sems = {}
        for name in sorted({v[0] for v in dma_done.values()}):
            sems[name] = ctx.enter_context(nc.semaphore(name))
        d_y = [ctx.enter_context(nc.semaphore(f"d_y{s}")) for s in range(2)]

        block = ctx.enter_context(nc.Block(no_gpsimd_drain=True))

        # ---------------- SP: all input DMAs (FIFO issue ring) -------------
        @block.sync
        def _(sync):
            w13_last = [0] * W13S  # s_pe tick of previous reader of the slot
            # slot 0 starts holding w13 tile (0,0), streamed in quarters from
            # the ACT queue — gate its first regular reload on those readers
            w13_last[0] = T_pu[0, 0, NBLK[0] - 1]
            w2_last = [0] * W2S
            for key in dma_seq:
                kind, e, j = key
                sem_name, _thr = dma_done[key]
                sem = sems[sem_name]
                NB = NBS[e] if kind in ("xa", "xb", "xk") else None
                if kind == "xk":
                    # expert-0 prologue: XCH k-tiles of token block 0
                    k0 = j * XCH
                    if k0 + XCH <= KH:
                        dst = xbufA[0][:, k0 : k0 + XCH, 0 : NBS[0]]
                    else:
                        dst = xbufB[:, k0 - KH : k0 - KH + XCH, 0 : NBS[0]]
                    sync.dma_start(
                        dst,
                        xT[0, k0 * P : (k0 + XCH) * P, 0 : NBS[0]].rearrange(
                            "(ko p) c -> p ko c", p=P
                        ),
                    ).then_inc(sem, 16)
                elif kind == "xa":
                    if e >= 2 and j == 0:
                        sync.wait_ge(s_pe, T_pu[e - 2, NI - 1, NBLK[e - 2] - 1])
                    sync.dma_start(
                        xbufA[e % 2][:, :, j * NB : (j + 1) * NB],
                        xT[e, : KH * P, j * NB : (j + 1) * NB].rearrange(
                            "(ko p) c -> p ko c", p=P
                        ),
                    ).then_inc(sem, 16)
                elif kind == "xb":
                    if e >= 1 and j == 0:
                        sync.wait_ge(s_pe, T_pu[e - 1, NI - 1, NBLK[e - 1] - 1])
                    sync.dma_start(
                        xbufB[:, :, j * NB : (j + 1) * NB],
                        xT[e, KH * P :, j * NB : (j + 1) * NB].rearrange(
                            "(ko p) c -> p ko c", p=P
                        ),
                    ).then_inc(sem, 16)
                elif kind == "w13":
                    s = j % W13S
                    if w13_last[s]:
                        sync.wait_ge(s_pe, w13_last[s])
                    w13_last[s] = T_pu[e, j, NBLK[e] - 1]
                    sync.dma_start(
                        w13b[s][:],
                        w13s[e, j].rearrange("(ko p) g -> p ko g", p=P),
                    ).then_inc(sem, 16)
                else:
                    s = j % W2S
                    if w2_last[s]:
                        sync.wait_ge(s_pe, w2_last[s])
                    w2_last[s] = T_py[e, j, NBLK[e] - 1]
                    sync.dma_start(
                        w2b[s][:],
                        w2s[e, j].rearrange("(ko p) g -> p ko g", p=P),
                    ).then_inc(sem, 16)
            # w13 tile (0,0) streams via w13q, so slot 0's reuse-gate for the
            # first regular load is tracked above only from ("w13", e, j) keys.

        # ---------------- PE: all matmuls ----------------------------------
        @block.tensor
        def _(tensor):
            probe(tensor, "pe_start")
            pg_last = [0, 0]  # s_dve tick of previous reader of pg slot
            pu_last = [0, 0]  # s_dve tick of previous reader of pu slot
            py_last = [0, 0]  # s_dve tick of previous reader of py slot
            for e in range(NSEQ):
                NB = NBS[e]
                for i in range(NI):
                    for cb in range(NBLK[e]):
                        gs = (NBLK_TOT[e] * NI + i * NBLK[e] + cb) % 2
                        cs = slice(cb * NB, (cb + 1) * NB)
                        first_blk = e == 0 and i == 0 and cb == 0
                        if cb == 0 and not (e == 0 and i == 0):
                            sem, thr = dma_done[("w13", e, i)]
                            tensor.wait_ge(sems[sem], thr)
                        if i == 0 and not first_blk:
                            sem, thr = dma_done[("xa", e, cb)]
                            tensor.wait_ge(sems[sem], thr)
                        if pg_last[gs]:
                            tensor.wait_ge(s_dve, pg_last[gs])
                        pg_last[gs] = T_h[e, i, cb]
                        for k in range(KD):
                            if first_blk:
                                if k % (KD // NQ) == 0:
                                    sem, thr = dma_done[("w13g", 0, k // (KD // NQ))]
                                    tensor.wait_ge(sems[sem], thr)
                                if k % XCH == 0:
                                    sem, thr = dma_done[("xk", 0, k // XCH)]
                                    tensor.wait_ge(sems[sem], thr)
                            elif i == 0 and k == KH:
                                sem, thr = dma_done[("xb", e, cb)]
                                tensor.wait_ge(sems[sem], thr)
                            mm = tensor.matmul(
                                pg[gs][:, :NB],
                                w13b[i % W13S][:, k, 0:P],
                                xbufA[e % 2][:, k, cs]
                                if k < KH
                                else xbufB[:, k - KH, cs],
                                start=(k == 0),
                                stop=(k == KD - 1),
                            )
                        mm.then_inc(s_pe)
                        if pu_last[gs]:
                            tensor.wait_ge(s_dve, pu_last[gs])
                        pu_last[gs] = T_h[e, i, cb]
                        for k in range(KD):
                            if first_blk and k % (KD // NQ) == 0:
                                sem, thr = dma_done[("w13u", 0, k // (KD // NQ))]
                                tensor.wait_ge(sems[sem], thr)
                            mm = tensor.matmul(
                                pu[gs][:, :NB],
                                w13b[i % W13S][:, k, P : 2 * P],
                                xbufA[e % 2][:, k, cs]
                                if k < KH
                                else xbufB[:, k - KH, cs],
                                start=(k == 0),
                                stop=(k == KD - 1),
                            )
                        mm.then_inc(s_pe)
                        probe(tensor, ("p1", e, i, cb))
                for d in range(ND):
                    for cb in range(NBLK[e]):
                        ys = (NBLK_TOT[e] * ND + d * NBLK[e] + cb) % 2
                        cs = slice(cb * NB, (cb + 1) * NB)
                        if cb == 0:
                            sem, thr = dma_done[("w2", e, d)]
                            tensor.wait_ge(sems[sem], thr)
                        if py_last[ys]:
                            tensor.wait_ge(s_dve, py_last[ys])
                        py_last[ys] = T_yc[e, d, cb]
                        for k in range(NI):
                            if d == 0:
                                # start the down-proj as h tiles land
                                tensor.wait_ge(s_dve, T_h[e, k, cb])
                            mm = tensor.matmul(
                                py[ys][:, :NB],
                                w2b[d % W2S][:, k, :],
                                hbuf[:, k, cs],
                                start=(k == 0),
                                stop=(k == NI - 1),
                            )
                        mm.then_inc(s_pe)
                        probe(tensor, ("p2", e, d, cb))

        # ---------------- ACT: silu + output DMAs --------------------------
        @block.scalar
        def _(scalar):
            import concourse.mybir as mybir

            # prologue: stream w13 tile (0,0) as gate then up quarters from
            # this queue, in parallel with SP's x chunks (separate sequencers
            # -> both issue streams start immediately)
            kq = KD // NQ
            for half, kind in ((0, "w13g"), (1, "w13u")):
                for q in range(NQ):
                    sem_name, _thr = dma_done[(kind, 0, q)]
                    scalar.dma_start(
                        w13b[0][:, q * kq : (q + 1) * kq, half * P : (half + 1) * P],
                        w13s[0, 0, q * kq * P : (q + 1) * kq * P,
                             half * P : (half + 1) * P].rearrange(
                            "(ko p) g -> p ko g", p=P
                        ),
                    ).then_inc(sems[sem_name], 16)

            ss_last = [0, 0]  # s_dve tick of previous reader of ss slot
            for e in range(NSEQ):
                NB = NBS[e]
                for i in range(NI):
                    for cb in range(NBLK[e]):
                        gs = (NBLK_TOT[e] * NI + i * NBLK[e] + cb) % 2
                        scalar.wait_ge(s_pe, T_pg[e, i, cb])
                        if ss_last[gs]:
                            scalar.wait_ge(s_dve, ss_last[gs])
                        ss_last[gs] = T_h[e, i, cb]
                        scalar.activation(
                            ssb[gs][:, :NB],
                            pg[gs][:, :NB],
                            mybir.ActivationFunctionType.Sigmoid,
                        ).then_inc(s_act)
                for d in range(ND):
                    for cb in range(NBLK[e]):
                        ys = (NBLK_TOT[e] * ND + d * NBLK[e] + cb) % 2
                        scalar.wait_ge(s_dve, T_yc[e, d, cb])
                        scalar.dma_start(
                            y[e, d * P : (d + 1) * P, cb * NB : (cb + 1) * NB],
                            ysb[ys][:, :NB],
                        ).then_inc(d_y[ys], 16)

        # ---------------- DVE: h-mul + psum eviction -----------------------
        @block.vector
        def _(vector):
            ysb_cnt = [0, 0]  # completed y-DMA count per ysb slot
            ysb_pend = [0, 0]
            for e in range(NSEQ):
                NB = NBS[e]
                for i in range(NI):
                    for cb in range(NBLK[e]):
                        gs = (NBLK_TOT[e] * NI + i * NBLK[e] + cb) % 2
                        cs = slice(cb * NB, (cb + 1) * NB)
                        vector.wait_ge(s_act, T_ss[e, i, cb])
                        vector.wait_ge(s_pe, T_pu[e, i, cb])
                        # h = (sigmoid(g) * u) * g  — one PSUM operand per op
                        vector.tensor_mul(
                            ssb[gs][:, :NB], ssb[gs][:, :NB], pu[gs][:, :NB]
                        ).then_inc(s_dve)
                        vector.wait_ge(s_dve, T_h[e, i, cb] - 1)
                        vector.tensor_mul(
                            hbuf[:, i, cs], ssb[gs][:, :NB], pg[gs][:, :NB]
                        ).then_inc(s_dve)
                for d in range(ND):
                    for cb in range(NBLK[e]):
                        ys = (NBLK_TOT[e] * ND + d * NBLK[e] + cb) % 2
                        vector.wait_ge(s_pe, T_py[e, d, cb])
                        if ysb_pend[ys]:
                            vector.wait_ge(d_y[ys], 16 * ysb_pend[ys])
                        ysb_cnt[ys] += 1
                        ysb_pend[ys] = ysb_cnt[ys]
                        vector.tensor_copy(
                            ysb[ys][:, :NB], py[ys][:, :NB]
                        ).then_inc(s_dve)

    return nc


def build_moe(EPC, D, I, CS, NBS, probe=None):
    import concourse.bass as bass
    import concourse.mybir as mybir

    f32 = mybir.dt.float32
    bf16 = mybir.dt.bfloat16
    NI = I // P
    ND = D // P
    CMAX = max(CS)

    nc = bass.Bass()
    xT = nc.dram_tensor("xT", [EPC, D, CMAX], bf16, kind="ExternalInput")
    w13s = nc.dram_tensor("w13s", [EPC, NI, D, 2 * P], bf16, kind="ExternalInput")
    w2s = nc.dram_tensor("w2s", [EPC, ND, I, P], bf16, kind="ExternalInput")
    y = nc.dram_tensor("y", [EPC, D, CMAX], f32, kind="ExternalOutput")
    emit_moe(nc, xT, w13s, w2s, y, EPC, D, I, CS, NBS, probe=probe)
    return nc


# --------------------------------------------------------------------------
# Host side
# --------------------------------------------------------------------------

def _route(x, gate_w, top_k):
    """float64 router: softmax over gate logits, top-k (set), renormalize."""
    logits = x.astype(np.float64) @ gate_w.astype(np.float64).T
    logits -= logits.max(axis=-1, keepdims=True)
    p = np.exp(logits)
    p /= p.sum(axis=-1, keepdims=True)
    ids = np.argpartition(-p, top_k - 1, axis=-1)[:, :top_k]  # [T, K]
    w = np.take_along_axis(p, ids, axis=-1)
    w = w / w.sum(axis=-1, keepdims=True)
    return ids, w


def _pack_weights(w13, w2, np_bf16):
    E, twoI, D = w13.shape
    I = twoI // 2
    NI = I // P
    ND = D // P
    g = w13[:, :I, :].reshape(E, NI, P, D).transpose(0, 1, 3, 2)  # [E,NI,D,P]
    u = w13[:, I:, :].reshape(E, NI, P, D).transpose(0, 1, 3, 2)
    w13s = np.concatenate([g, u], axis=-1).astype(np_bf16)  # [E, NI, D, 2P]
    w2s = w2.reshape(E, ND, P, I).transpose(0, 1, 3, 2).astype(np_bf16)
    return np.ascontiguousarray(w13s), np.ascontiguousarray(w2s)


def _capacity(cmax):
    """Token capacity (multiple-of-4 padded) and block size for a class."""
    nblk = max(1, math.ceil(cmax / 512))
    NB = math.ceil(cmax / nblk / 4) * 4
    return NB * nblk, NB


def prepare(hidden_states, gate_w, w13, w2, top_k):
    """Host routing + dispatch + device-layout packing.

    Returns (nc, in_maps, meta) where meta carries what combine() needs.
    """
    import concourse.mybir as mybir

    np_bf16 = mybir.dt.np(mybir.dt.bfloat16)

    x = np.ascontiguousarray(np.asarray(hidden_states, dtype=np.float32))
    gate_w = np.asarray(gate_w, dtype=np.float32)
    w13 = np.asarray(w13, dtype=np.float32)
    w2 = np.asarray(w2, dtype=np.float32)
    K = int(top_k)

    T, D = x.shape
    E = gate_w.shape[0]
    I = w2.shape[2]
    EPC = E // N_CORES

    topk_ids, topk_w = _route(x, gate_w, K)

    # dispatch: group (token, weight) pairs by expert
    flat_e = topk_ids.ravel()
    flat_t = np.repeat(np.arange(T), K)
    flat_w = topk_w.ravel()
    order = np.argsort(flat_e, kind="stable")
    sorted_t = flat_t[order]
    sorted_w = flat_w[order]
    counts = np.bincount(flat_e, minlength=E)
    offs = np.zeros(E + 1, np.int64)
    np.cumsum(counts, out=offs[1:])

    # slot classes: 8 highest-count experts -> slot 0, rest -> slot 1
    rank = np.argsort(-counts, kind="stable")  # expert ids, desc count
    slot_experts = [rank[:N_CORES], rank[N_CORES:]]  # [slot][core] -> expert
    CS, NBS = [], []
    for cls in slot_experts:
        cmax = max(int(counts[cls].max()), 16)
        C, NB = _capacity(cmax)
        CS.append(C)
        NBS.append(NB)
    CMAX = max(CS)

    xbf = x.astype(np_bf16)
    xT_all = np.zeros((E, D, CMAX), np_bf16)
    for e in range(E):
        idx = sorted_t[offs[e] : offs[e + 1]]
        if len(idx):
            xT_all[e, :, : len(idx)] = xbf[idx].T
    w13s, w2s = _pack_weights(w13, w2, np_bf16)

    nc = build_moe(EPC, D, I, CS, NBS)
    in_maps = []
    for m in range(N_CORES):
        exps = [slot_experts[s][m] for s in range(EPC)]
        in_maps.append(
            {
                "xT": np.ascontiguousarray(xT_all[exps]),
                "w13s": np.ascontiguousarray(w13s[exps]),
                "w2s": np.ascontiguousarray(w2s[exps]),
            }
        )
    meta = dict(
        T=T, D=D, E=E, EPC=EPC, CS=CS, NBS=NBS,
        slot_experts=slot_experts,
        sorted_t=sorted_t, sorted_w=sorted_w, offs=offs,
        n_mm_per_core=sum(
            (CS[s] // NBS[s]) * ((I // P) * 2 * (D // P) + (D // P) * (I // P))
            for s in range(EPC)
        ),
    )
    return nc, in_maps, meta


def combine(results, meta):
    """Weighted scatter-add of per-expert outputs back to [T, D]."""
    T, D, EPC = meta["T"], meta["D"], meta["EPC"]
    slot_experts = meta["slot_experts"]
    sorted_t, sorted_w, offs = meta["sorted_t"], meta["sorted_w"], meta["offs"]
    out = np.zeros((T, D), np.float32)
    for s in range(EPC):
        for m in range(N_CORES):
            e = int(slot_experts[s][m])
            idx = sorted_t[offs[e] : offs[e + 1]]
            if len(idx) == 0:
                continue
            wgt = sorted_w[offs[e] : offs[e + 1]].astype(np.float32)
            ye = results[m]["y"][s]  # [D, CMAX] fp32
            out[idx] += (ye[:, : len(idx)] * wgt[None, :]).T
    return out


def kernel(hidden_states, gate_w, w13, w2, top_k):
    from concourse.bass_utils import run_bass_kernel_spmd

    nc, in_maps, meta = prepare(hidden_states, gate_w, w13, w2, top_k)
    trace = bool(int(os.environ.get("MOE_TRACE", "0")))
    try:
        res = run_bass_kernel_spmd(
            nc, in_maps, core_ids=list(range(N_CORES)), trace=trace
        )
    except Exception:
        # one retry — transient NRT device errors (e.g. a wedged core from a
        # previous aborted run) usually clear on re-execution
        import time as _time

        _time.sleep(5)
        res = run_bass_kernel_spmd(
            nc, in_maps, core_ids=list(range(N_CORES)), trace=trace
        )
    global LAST_EXEC_NS
    LAST_EXEC_NS = res.exec_time_ns
    return combine(res.results, meta)
